# revision 23
# baseline (speedup 1.0000x reference)
"""Trainium2 Bass kernel for nn_AutoregressiveRoutingHead.

Model (per batch row b):
    tok_in = [START, tgt[0..6]]                       # teacher forcing, START=5
    x_t    = emb[tok_in[t]]                           # (HID,)
    gi     = x_t @ W_ih.T + b_ih                      # (768,)
    gh     = h @ W_hh.T + b_hh                        # (768,)
    r = sigmoid(gi_r + gh_r); z = sigmoid(gi_z + gh_z)
    n = tanh(gi_n + r * gh_n)
    h' = (1-z)*n + z*h = n - z*(n - h)
    logits_t = h' @ W_out.T + b_out                   # (5,)

Strategy: pure data parallel over batch (65536 -> 8 x 8192). On each core the
hidden state lives TRANSPOSED (latent on partitions, batch on free dim).
Four batch chunks ("parities") are processed in lockstep per step so every
engine always has independent work and the PE never idles (keeps the HAM
clock-gate at 8/8).  All host-derivable constants are precomputed on the
host: h0 arrives pre-transposed in f16, the per-step token one-hots arrive
as f16 (replicated into the four PE row-groups), and the n-gate input-bias
gi_n[tok] arrives pre-gathered in f16 so it is consumed by a 2x-rate DVE add
instead of a PSUM-rate add + extra matmuls.  The r/z input biases ride into
PSUM via a K=8 one-hot matmul packed 4-way into PE row groups.  Step-0
(constant START token) input biases are folded into the sigmoid/tanh-side
activations as per-partition scalars.  Logits for the 4 parities land in one
PSUM bank at partition offsets 32*par, so one scalar copy evacuates 4 chunks
of logits at a time.
"""

import numpy as np

import concourse.bass as bass
import concourse.mybir as mybir
import concourse.tile as tile
from concourse import bacc, bass_utils

F32 = mybir.dt.float32
F16 = mybir.dt.float16
AF = mybir.ActivationFunctionType
ALU = mybir.AluOpType

N_CORES = 8
B = 65536
L = 8
LATENT = 256
HID = 128
NTOK = 5
START = NTOK
G = 3 * LATENT  # 768 gate rows
KC = LATENT // 128  # 2 contraction chunks
B_CORE = B // N_CORES
N_PAR = 4  # chunks processed in lockstep


def build_program(b_core=B_CORE, n_b=512, n_par=N_PAR, use_bhh_n=False,
                  use_bout=False):
    """Build + compile the per-core Bass program (SPMD: same program, 8 cores)."""
    nc = bacc.Bacc("TRN2", target_bir_lowering=False, debug=False)
    DT = F16
    n_chunks = b_core // n_b
    n_groups = n_chunks // n_par
    assert n_chunks % n_par == 0

    # ---- DRAM I/O ----------------------------------------------------------
    latT = nc.dram_tensor("latT", [LATENT, b_core], DT, kind="ExternalInput").ap()
    # one-hot of input tokens for steps 1..7: oh[v, s, b] = (tok_in[b, s+1]==v)
    ohd = nc.dram_tensor("ohd", [8, L - 1, b_core], DT, kind="ExternalInput").ap()
    # pre-gathered n-part input bias: gin[s, q, b] = gi_n[tok_in[b, s+1], q]
    gind = nc.dram_tensor("gind", [L - 1, LATENT, b_core], DT,
                          kind="ExternalInput").ap()
    # rz-part of the gi table (rows: token ids, cols: 512 gate cols), 4-replic
    girz = nc.dram_tensor("girz", [8, 512], DT, kind="ExternalInput").ap()
    # START-token gi column (for step-0 per-partition biases): [128, 6]
    gist = nc.dram_tensor("gist", [128, 6], F32, kind="ExternalInput").ap()
    whhT = nc.dram_tensor("whhT", [LATENT, G], DT, kind="ExternalInput").ap()
    woutT = nc.dram_tensor("woutT", [LATENT, 32], DT, kind="ExternalInput").ap()
    bhhn = bout = None
    if use_bhh_n:
        bhhn = nc.dram_tensor("bhhn", [1, LATENT], DT, kind="ExternalInput").ap()
    if use_bout:
        bout = nc.dram_tensor("bout", [1, 32], DT, kind="ExternalInput").ap()
    outT = nc.dram_tensor("outT", [L, NTOK, b_core], DT, kind="ExternalOutput").ap()

    with tile.TileContext(nc) as tc:
        with tc.tile_pool(name="singles", bufs=1) as singles, \
             tc.tile_pool(name="chunk_in", bufs=2) as c_pool, \
             tc.tile_pool(name="hpool", bufs=3) as h_pool, \
             tc.tile_pool(name="gates", bufs=1) as g_pool, \
             tc.tile_pool(name="ps_rz", bufs=1, space="PSUM") as ps_rz, \
             tc.tile_pool(name="ps_hn", bufs=1, space="PSUM") as ps_hn, \
             tc.tile_pool(name="ps_lg", bufs=1, space="PSUM") as ps_lg:

            # ---- constants / weights in SBUF -------------------------------
            whh_sb = singles.tile([128, KC, G], DT, tag="whh")
            nc.sync.dma_start(whh_sb, whhT.rearrange("(kc p) n -> p kc n", p=128))
            # wout is padded to 32 output rows so the 4 parities' logits MMs
            # cover all 128 partitions of the lg PSUM tile (no memset needed).
            wout_sb = singles.tile([128, KC, 32], DT, tag="wout")
            nc.sync.dma_start(wout_sb, woutT.rearrange("(kc p) n -> p kc n", p=128))
            girz_sb = singles.tile([128, 512], DT, tag="girz")
            for gq in range(4):
                nc.sync.dma_start(girz_sb[32 * gq:32 * gq + 8], girz)
            gist_sb = singles.tile([128, 6], F32, tag="gist")
            nc.sync.dma_start(gist_sb, gist)
            bhhn_sb = bout_sb = ones_row = None
            if use_bhh_n or use_bout:
                ones_row = singles.tile([1, n_b], DT, tag="ones_row")
                nc.vector.memset(ones_row, 1.0)
            if use_bhh_n:
                bhhn_sb = singles.tile([1, LATENT], DT, tag="bhhn")
                nc.sync.dma_start(bhhn_sb, bhhn)
            if use_bout:
                bout_sb = singles.tile([1, 32], DT, tag="bout")
                nc.sync.dma_start(bout_sb, bout)

            def chunk_dmas(g, par):
                """Load h0 / one-hot for chunk (g, par)."""
                c = g * n_par + par
                cs = slice(c * n_b, (c + 1) * n_b)
                h = h_pool.tile([128, KC, n_b], DT, tag=f"h{par}", name="h0")
                nc.sync.dma_start(
                    h, latT.rearrange("(k p) b -> p k b", p=128)[:, :, cs])
                oh = c_pool.tile([128, L - 1, n_b], DT, tag=f"oh{par}", name="oh")
                for gq in range(4):
                    nc.sync.dma_start(oh[32 * gq:32 * gq + 8], ohd[:, :, cs])
                return cs, oh, h

            def gin_dma(g, par, t):
                """Prefetch the n-gate input bias for (chunk, step t>=1)."""
                c = g * n_par + par
                cs = slice(c * n_b, (c + 1) * n_b)
                gt = c_pool.tile([128, KC, n_b], DT, tag=f"gin{par}", bufs=4,
                                 name="gin_t")
                nc.sync.dma_start(
                    gt, gind[t - 1].rearrange("(k p) b -> p k b", p=128)[:, :, cs])
                return gt

            def step_mms(t, par, oh, h):
                """All matmuls for (parity, step) except logits.

                r/z/n gate pre-acts land in three 2-bank PSUM pair tiles so
                the sigmoids and the r*gh_n multiply run as single FD=1024
                instructions."""
                rp = ps_rz.tile([128, 2, n_b], F32, tag="rp", name="rp")
                zp = ps_rz.tile([128, 2, n_b], F32, tag="zp", name="zp")
                hn = ps_hn.tile([128, 2, n_b], F32, tag="hn", name="hn")
                rz_out = [rp[:, 0, :], rp[:, 1, :], zp[:, 0, :], zp[:, 1, :]]
                for m in range(4):
                    for k in range(KC):
                        nc.tensor.matmul(
                            rz_out[m],
                            lhsT=whh_sb[:, k, m * 128:(m + 1) * 128],
                            rhs=h[:, k, :],
                            start=(k == 0),
                            stop=(k == KC - 1) and (t == 0))
                for j in range(KC):
                    m = 4 + j
                    for k in range(KC):
                        nc.tensor.matmul(
                            hn[:, j, :],
                            lhsT=whh_sb[:, k, m * 128:(m + 1) * 128],
                            rhs=h[:, k, :],
                            start=(k == 0),
                            stop=(k == KC - 1) and not use_bhh_n)
                    if use_bhh_n:
                        nc.tensor.matmul(
                            hn[:, j, :], lhsT=bhhn_sb[:, j * 128:(j + 1) * 128],
                            rhs=ones_row, start=False, stop=True)
                if t > 0:
                    # K=8 one-hot matmuls (input biases), packed 4-way into PE
                    # row groups. Emitted last so they are 2+ us past the
                    # previous parity's sigmoid (which frees these banks).
                    for m in range(4):
                        nc.tensor.matmul(
                            rz_out[m],
                            lhsT=girz_sb[32 * m:32 * m + 8, m * 128:(m + 1) * 128],
                            rhs=oh[32 * m:32 * m + 8, t - 1, :],
                            start=False, stop=True,
                            tile_position=(32 * m, 0))
                return rp, zp, hn

            def sig_stage(t, par, rp, zp):
                """r/z sigmoids for (parity, step) — emitted immediately so
                the banks free up for the next parity without queuing behind
                this parity's tanh."""
                rz_sig = g_pool.tile([128, 4, n_b], DT, tag=f"rzs{par}",
                                     name="rz_sig")
                if t == 0:
                    # per-m START biases differ, so four FD=512 activations
                    for m, src in enumerate(
                            (rp[:, 0, :], rp[:, 1, :], zp[:, 0, :], zp[:, 1, :])):
                        nc.scalar.activation(
                            rz_sig[:, m, :], src, AF.Sigmoid,
                            bias=gist_sb[:, m:m + 1])
                else:
                    nc.scalar.activation(rz_sig[:, 0:2, :], rp, AF.Sigmoid)
                    nc.scalar.activation(rz_sig[:, 2:4, :], zp, AF.Sigmoid)
                return rz_sig

            def chain_stage(t, par, rz_sig, hn, gin_t, h):
                """Deferred elementwise chain; returns h_new."""
                r = rz_sig[:, 0:KC, :]
                z = rz_sig[:, KC:4, :]
                p = g_pool.tile([128, KC, n_b], DT, tag=f"p{par}", name="p")
                nc.vector.tensor_mul(p, r, hn)
                npre = g_pool.tile([128, KC, n_b], DT, tag=f"np{par}", name="npre")
                if t == 0:
                    for j in range(KC):
                        nc.vector.tensor_scalar_add(
                            npre[:, j, :], p[:, j, :], gist_sb[:, 4 + j:5 + j])
                else:
                    nc.vector.tensor_add(npre, p, gin_t)
                nt = g_pool.tile([128, KC, n_b], DT, tag=f"nt{par}", name="nt")
                nc.scalar.activation(nt, npre, AF.Tanh)
                u = g_pool.tile([128, KC, n_b], DT, tag=f"u{par}", name="u")
                nc.vector.tensor_tensor(u, nt, h, ALU.subtract)
                e = g_pool.tile([128, KC, n_b], DT, tag=f"e{par}", name="e")
                nc.vector.tensor_mul(e, z, u)
                h_new = h_pool.tile([128, KC, n_b], DT, tag=f"h{par}", name="h_new")
                nc.vector.tensor_tensor(h_new, nt, e, ALU.subtract)
                return h_new

            def logits_mm(sl, par, h_new, lg_ps):
                """Logits for (parity, step); sl = step slot (0/1) in the
                2-step lg pair tile. M=32 (padded wout) so the 4 parities
                cover all 128 partitions."""
                out = lg_ps[32 * par:32 * par + 32, sl, :]
                for k in range(KC):
                    nc.tensor.matmul(
                        out, lhsT=wout_sb[:, k, :], rhs=h_new[:, k, :],
                        start=(k == 0), stop=(k == KC - 1) and not use_bout,
                        tile_position=(0, 32 * par))
                if use_bout:
                    nc.tensor.matmul(
                        out, lhsT=bout_sb, rhs=ones_row, start=False, stop=True,
                        tile_position=(0, 32 * par))

            def logits_out(t, g, lg_ps):
                """Evacuate one 2-step x 4-parity lg pair tile (steps t-1, t)."""
                lg_sb = g_pool.tile([128, 2, n_b], DT, tag="lgsb", bufs=2,
                                    name="lg_sb")
                nc.scalar.copy(lg_sb, lg_ps)
                for gq in range(n_par):
                    cb = (g * n_par + gq) * n_b
                    nc.sync.dma_start(
                        outT[t - 1:t + 1, :, cb:cb + n_b].rearrange(
                            "s v b -> v s b")[0:NTOK],
                        lg_sb[32 * gq:32 * gq + NTOK, :, :])

            # ---- main loop: one continuous software pipeline over all
            # (round, step) wall-slots; parity par works chunk 4*round+par.
            # Logits matmuls are deferred 2 parity-slots so h_new is never on
            # the tensor queue's critical path; next round's chunk DMAs are
            # prefetched at step 5 of the current round. ---------------------
            states = [chunk_dmas(0, par) for par in range(n_par)]
            nxt = [None] * n_par
            gins = [[None] * (n_groups * L + 4) for _ in range(n_par)]
            for par in range(n_par):
                for s in (1, 2):
                    gins[par][s] = gin_dma(0, par, s)
            pend = []  # deferred logits mms: (ws, par, h_new)
            ew_pend = []  # deferred elementwise chains: (ws, par, sig, hn, h)
            lgs = {}

            def pop_logits():
                ws, ppar, ph = pend.pop(0)
                logits_mm(ws % 2, ppar, ph, lgs[ws // 2])
                if ppar == n_par - 1 and ws % 2 == 1:
                    logits_out(ws % L, ws // L, lgs[ws // 2])
                    del lgs[ws // 2]

            def pop_chain():
                ws, ppar, sig, hn, h = ew_pend.pop(0)
                h_new = chain_stage(ws % L, ppar, sig, hn, gins[ppar][ws], h)
                pend.append((ws, ppar, h_new))
                cs, oh, _ = states[ppar]
                states[ppar] = (cs, oh, h_new)

            for ws in range(n_groups * L):
                g, t = ws // L, ws % L
                if ws % 2 == 0:
                    lgs[ws // 2] = ps_lg.tile([128, 2, n_b], F32, tag="lg",
                                              name=f"lg{t}")
                for par in range(n_par):
                    cs, oh, h = states[par]
                    rp, zp, hn = step_mms(t, par, oh, h)
                    sig = sig_stage(t, par, rp, zp)
                    ew_pend.append((ws, par, sig, hn, h))
                    if len(ew_pend) > 1:
                        pop_chain()
                    if len(pend) > 2:
                        pop_logits()
                    s = ws + 3
                    if s < n_groups * L and s % L != 0:
                        gins[par][s] = gin_dma(s // L, par, s % L)
                    if t == 5 and g + 1 < n_groups:
                        nxt[par] = chunk_dmas(g + 1, par)
                if t == L - 1:
                    while ew_pend:
                        pop_chain()
                    states, nxt = nxt, [None] * n_par
            while ew_pend:
                pop_chain()
            while pend:
                pop_logits()

    nc.compile()
    return nc


def make_in_maps(latent_context, target_sequence, emb_table, W_ih, W_hh,
                 b_ih, b_hh, W_out, b_out, b_core=B_CORE):
    """Shard + lay out the inputs for each core. Host-side layout/lookup prep."""
    lat = np.asarray(latent_context, dtype=np.float32)
    tok = np.asarray(target_sequence)
    emb = np.asarray(emb_table, dtype=np.float64)
    W_ih = np.asarray(W_ih, dtype=np.float64)
    b_ih = np.asarray(b_ih, dtype=np.float64)
    b_hh = np.asarray(b_hh, dtype=np.float64)

    # gi table: gi[v] = emb[v] @ W_ih.T + b_ih (+ b_hh on the rz part)
    gi = emb @ W_ih.T + b_ih
    gi[:, :512] += b_hh[:512]
    gi = gi.astype(np.float32)

    girz = np.zeros((8, 512), np.float16)
    girz[:NTOK] = gi[:NTOK, :512].astype(np.float16)
    gist = np.ascontiguousarray(
        gi[START].reshape(6, 128).T.astype(np.float32))  # [128, 6]

    whhT = np.ascontiguousarray(
        np.asarray(W_hh, np.float32).T.astype(np.float16))
    woutT = np.zeros((LATENT, 32), np.float16)
    woutT[:, :NTOK] = np.asarray(W_out, np.float32).T.astype(np.float16)
    b_out = np.asarray(b_out, dtype=np.float32)

    latT_all = lat.astype(np.float16).T  # [256, B]
    # input tokens for steps 1..7 are tgt[:, 0..6]
    ti = tok[:, :L - 1].astype(np.int64)  # [B, 7]
    oh_all = (ti[:, :, None] == np.arange(8)).astype(np.float16)  # [B,7,8]
    oh_all = np.ascontiguousarray(oh_all.transpose(2, 1, 0))  # [8,7,B]
    gi_n16 = gi[:, 512:].astype(np.float16)  # [6, 256]
    gin_all = gi_n16[ti]  # [B, 7, 256]
    gin_all = np.ascontiguousarray(gin_all.transpose(1, 2, 0))  # [7,256,B]

    n_cores_eff = lat.shape[0] // b_core
    in_maps = []
    for i in range(n_cores_eff):
        sl = slice(i * b_core, (i + 1) * b_core)
        m = {
            "latT": np.ascontiguousarray(latT_all[:, sl]),
            "ohd": np.ascontiguousarray(oh_all[:, :, sl]),
            "gind": np.ascontiguousarray(gin_all[:, :, sl]),
            "girz": girz,
            "gist": gist,
            "whhT": whhT,
            "woutT": woutT,
        }
        if np.any(b_hh[512:]):
            m["bhhn"] = np.ascontiguousarray(
                b_hh[512:].reshape(1, LATENT).astype(np.float16))
        if np.any(b_out):
            bo = np.zeros((1, 32), np.float16)
            bo[0, :NTOK] = b_out.astype(np.float16)
            m["bout"] = bo
        in_maps.append(m)
    return in_maps


_PROGRAM_CACHE = {}


def _get_program(b_core, use_bhh_n, use_bout):
    key = (b_core, use_bhh_n, use_bout)
    if key not in _PROGRAM_CACHE:
        _PROGRAM_CACHE[key] = build_program(
            b_core=b_core, use_bhh_n=use_bhh_n, use_bout=use_bout)
    return _PROGRAM_CACHE[key]


def run(inputs, trace=False, b_core=B_CORE, **_ignored):
    in_maps = make_in_maps(b_core=b_core, **inputs)
    use_bhh_n = "bhhn" in in_maps[0]
    use_bout = "bout" in in_maps[0]
    nc = _get_program(b_core, use_bhh_n, use_bout)
    core_ids = list(range(len(in_maps)))
    res = bass_utils.run_bass_kernel_spmd(nc, in_maps, core_ids, trace=trace)
    outs = []
    for i in core_ids:
        o = res.results[i]["outT"]  # (L, NTOK, b_core) f16
        outs.append(np.ascontiguousarray(
            np.transpose(o, (2, 0, 1)).astype(np.float32)))
    return np.concatenate(outs, axis=0), res


def kernel(**inputs) -> np.ndarray:
    out, _ = run(inputs, trace=False)
    return out


# revision 24
# speedup vs baseline: 1.2114x; 1.2114x over previous
"""Trainium2 Bass kernel for nn_AutoregressiveRoutingHead.

Model (per batch row b):
    tok_in = [START, tgt[0..6]]                       # teacher forcing, START=5
    x_t    = emb[tok_in[t]]                           # (HID,)
    gi     = x_t @ W_ih.T + b_ih                      # (768,)
    gh     = h @ W_hh.T + b_hh                        # (768,)
    r = sigmoid(gi_r + gh_r); z = sigmoid(gi_z + gh_z)
    n = tanh(gi_n + r * gh_n)
    h' = (1-z)*n + z*h = n - z*(n - h)
    logits_t = h' @ W_out.T + b_out                   # (5,)

Strategy: pure data parallel over batch (65536 -> 8 x 8192). On each core the
hidden state lives TRANSPOSED (latent on partitions, batch on free dim).
Four batch chunks ("parities") are processed in lockstep per step so every
engine always has independent work and the PE never idles (keeps the HAM
clock-gate at 8/8).  All host-derivable constants are precomputed on the
host: h0 arrives pre-transposed in f16, the per-step token one-hots arrive
as f16 (replicated into the four PE row-groups), and the n-gate input-bias
gi_n[tok] arrives pre-gathered in f16 so it is consumed by a 2x-rate DVE add
instead of a PSUM-rate add + extra matmuls.  The r/z input biases ride into
PSUM via a K=8 one-hot matmul packed 4-way into PE row groups.  Step-0
(constant START token) input biases are folded into the sigmoid/tanh-side
activations as per-partition scalars.  Logits for the 4 parities land in one
PSUM bank at partition offsets 32*par, so one scalar copy evacuates 4 chunks
of logits at a time.
"""

import numpy as np

import concourse.bass as bass
import concourse.mybir as mybir
import concourse.tile as tile
from concourse import bacc, bass_utils

F32 = mybir.dt.float32
F16 = mybir.dt.float16
AF = mybir.ActivationFunctionType
ALU = mybir.AluOpType

N_CORES = 8
B = 65536
L = 8
LATENT = 256
HID = 128
NTOK = 5
START = NTOK
G = 3 * LATENT  # 768 gate rows
KC = LATENT // 128  # 2 contraction chunks
B_CORE = B // N_CORES
N_PAR = 4  # chunks processed in lockstep


def build_program(b_core=B_CORE, n_b=512, n_par=N_PAR, use_bhh_n=False,
                  use_bout=False):
    """Build + compile the per-core Bass program (SPMD: same program, 8 cores)."""
    nc = bacc.Bacc("TRN2", target_bir_lowering=False, debug=False)
    DT = F16
    n_chunks = b_core // n_b
    n_groups = n_chunks // n_par
    assert n_chunks % n_par == 0

    # ---- DRAM I/O ----------------------------------------------------------
    latT = nc.dram_tensor("latT", [LATENT, b_core], DT, kind="ExternalInput").ap()
    # one-hot of input tokens for steps 1..7: oh[v, s, b] = (tok_in[b, s+1]==v)
    ohd = nc.dram_tensor("ohd", [8, L - 1, b_core], DT, kind="ExternalInput").ap()
    # pre-gathered n-part input bias: gin[s, q, b] = gi_n[tok_in[b, s+1], q]
    gind = nc.dram_tensor("gind", [L - 1, LATENT, b_core], DT,
                          kind="ExternalInput").ap()
    # rz-part of the gi table (rows: token ids, cols: 512 gate cols), 4-replic
    girz = nc.dram_tensor("girz", [8, 512], DT, kind="ExternalInput").ap()
    # START-token gi column (for step-0 per-partition biases): [128, 6]
    gist = nc.dram_tensor("gist", [128, 6], F32, kind="ExternalInput").ap()
    whhT = nc.dram_tensor("whhT", [LATENT, G], DT, kind="ExternalInput").ap()
    woutT = nc.dram_tensor("woutT", [LATENT, 32], DT, kind="ExternalInput").ap()
    bhhn = bout = None
    if use_bhh_n:
        bhhn = nc.dram_tensor("bhhn", [1, LATENT], DT, kind="ExternalInput").ap()
    if use_bout:
        bout = nc.dram_tensor("bout", [1, 32], DT, kind="ExternalInput").ap()
    outT = nc.dram_tensor("outT", [L, NTOK, b_core], DT, kind="ExternalOutput").ap()

    with tile.TileContext(nc) as tc:
        with tc.tile_pool(name="singles", bufs=1) as singles, \
             tc.tile_pool(name="chunk_in", bufs=2) as c_pool, \
             tc.tile_pool(name="hpool", bufs=3) as h_pool, \
             tc.tile_pool(name="gates", bufs=1) as g_pool, \
             tc.tile_pool(name="ps_rz", bufs=1, space="PSUM") as ps_rz, \
             tc.tile_pool(name="ps_hn", bufs=1, space="PSUM") as ps_hn, \
             tc.tile_pool(name="ps_lg", bufs=1, space="PSUM") as ps_lg:

            # ---- constants / weights in SBUF -------------------------------
            whh_sb = singles.tile([128, KC, G], DT, tag="whh")
            nc.sync.dma_start(whh_sb, whhT.rearrange("(kc p) n -> p kc n", p=128))
            # wout is padded to 32 output rows so the 4 parities' logits MMs
            # cover all 128 partitions of the lg PSUM tile (no memset needed).
            wout_sb = singles.tile([128, KC, 32], DT, tag="wout")
            nc.sync.dma_start(wout_sb, woutT.rearrange("(kc p) n -> p kc n", p=128))
            girz_sb = singles.tile([128, 512], DT, tag="girz")
            for gq in range(4):
                nc.sync.dma_start(girz_sb[32 * gq:32 * gq + 8], girz)
            gist_sb = singles.tile([128, 6], F32, tag="gist")
            nc.sync.dma_start(gist_sb, gist)
            bhhn_sb = bout_sb = ones_row = None
            if use_bhh_n or use_bout:
                ones_row = singles.tile([1, n_b], DT, tag="ones_row")
                nc.vector.memset(ones_row, 1.0)
            if use_bhh_n:
                bhhn_sb = singles.tile([1, LATENT], DT, tag="bhhn")
                nc.sync.dma_start(bhhn_sb, bhhn)
            if use_bout:
                bout_sb = singles.tile([1, 32], DT, tag="bout")
                nc.sync.dma_start(bout_sb, bout)

            def chunk_dmas(g, par):
                """Load h0 / one-hot for chunk (g, par)."""
                c = g * n_par + par
                cs = slice(c * n_b, (c + 1) * n_b)
                h = h_pool.tile([128, KC, n_b], DT, tag=f"h{par}", name="h0")
                nc.sync.dma_start(
                    h, latT.rearrange("(k p) b -> p k b", p=128)[:, :, cs])
                oh = c_pool.tile([128, L - 1, n_b], DT, tag=f"oh{par}", name="oh")
                for gq in range(4):
                    nc.sync.dma_start(oh[32 * gq:32 * gq + 8], ohd[:, :, cs])
                return cs, oh, h

            def gin_dma(g, par, t):
                """Prefetch the n-gate input bias for (chunk, step t>=1)."""
                c = g * n_par + par
                cs = slice(c * n_b, (c + 1) * n_b)
                gt = c_pool.tile([128, KC, n_b], DT, tag=f"gin{par}", bufs=4,
                                 name="gin_t")
                nc.sync.dma_start(
                    gt, gind[t - 1].rearrange("(k p) b -> p k b", p=128)[:, :, cs])
                return gt

            def step_mms(t, par, oh, h):
                """All matmuls for (parity, step) except logits.

                r/z/n gate pre-acts land in three 2-bank PSUM pair tiles so
                the sigmoids and the r*gh_n multiply run as single FD=1024
                instructions."""
                rp = ps_rz.tile([128, 2, n_b], F32, tag="rp", name="rp")
                zp = ps_rz.tile([128, 2, n_b], F32, tag="zp", name="zp")
                hn = ps_hn.tile([128, 2, n_b], F32, tag="hn", name="hn")
                rz_out = [rp[:, 0, :], rp[:, 1, :], zp[:, 0, :], zp[:, 1, :]]
                if t > 0:
                    # K=8 one-hot matmuls packed 4-way into PE row groups
                    for m in range(4):
                        nc.tensor.matmul(
                            rz_out[m],
                            lhsT=girz_sb[32 * m:32 * m + 8, m * 128:(m + 1) * 128],
                            rhs=oh[32 * m:32 * m + 8, t - 1, :],
                            start=True, stop=False,
                            tile_position=(32 * m, 0))
                for m in range(4):
                    for k in range(KC):
                        nc.tensor.matmul(
                            rz_out[m],
                            lhsT=whh_sb[:, k, m * 128:(m + 1) * 128],
                            rhs=h[:, k, :],
                            start=(k == 0) and (t == 0),
                            stop=(k == KC - 1))
                for j in range(KC):
                    m = 4 + j
                    for k in range(KC):
                        nc.tensor.matmul(
                            hn[:, j, :],
                            lhsT=whh_sb[:, k, m * 128:(m + 1) * 128],
                            rhs=h[:, k, :],
                            start=(k == 0),
                            stop=(k == KC - 1) and not use_bhh_n)
                    if use_bhh_n:
                        nc.tensor.matmul(
                            hn[:, j, :], lhsT=bhhn_sb[:, j * 128:(j + 1) * 128],
                            rhs=ones_row, start=False, stop=True)
                return rp, zp, hn

            def sig_stage(t, par, rp, zp):
                """r/z sigmoids for (parity, step) — emitted immediately so
                the banks free up for the next parity without queuing behind
                this parity's tanh."""
                rz_sig = g_pool.tile([128, 4, n_b], DT, tag=f"rzs{par}",
                                     name="rz_sig")
                if t == 0:
                    # per-m START biases differ, so four FD=512 activations
                    for m, src in enumerate(
                            (rp[:, 0, :], rp[:, 1, :], zp[:, 0, :], zp[:, 1, :])):
                        nc.scalar.activation(
                            rz_sig[:, m, :], src, AF.Sigmoid,
                            bias=gist_sb[:, m:m + 1])
                else:
                    nc.scalar.activation(rz_sig[:, 0:2, :], rp, AF.Sigmoid)
                    nc.scalar.activation(rz_sig[:, 2:4, :], zp, AF.Sigmoid)
                return rz_sig

            def chain_stage(t, par, rz_sig, hn, gin_t, h):
                """Deferred elementwise chain; returns h_new."""
                r = rz_sig[:, 0:KC, :]
                z = rz_sig[:, KC:4, :]
                p = g_pool.tile([128, KC, n_b], DT, tag=f"p{par}", name="p")
                nc.vector.tensor_mul(p, r, hn)
                npre = g_pool.tile([128, KC, n_b], DT, tag=f"np{par}", name="npre")
                if t == 0:
                    for j in range(KC):
                        nc.vector.tensor_scalar_add(
                            npre[:, j, :], p[:, j, :], gist_sb[:, 4 + j:5 + j])
                else:
                    nc.vector.tensor_add(npre, p, gin_t)
                nt = g_pool.tile([128, KC, n_b], DT, tag=f"nt{par}", name="nt")
                nc.scalar.activation(nt, npre, AF.Tanh)
                u = g_pool.tile([128, KC, n_b], DT, tag=f"u{par}", name="u")
                nc.vector.tensor_tensor(u, nt, h, ALU.subtract)
                e = g_pool.tile([128, KC, n_b], DT, tag=f"e{par}", name="e")
                nc.vector.tensor_mul(e, z, u)
                h_new = h_pool.tile([128, KC, n_b], DT, tag=f"h{par}", name="h_new")
                nc.vector.tensor_tensor(h_new, nt, e, ALU.subtract)
                return h_new

            def logits_mm(sl, par, h_new, lg_ps):
                """Logits for (parity, step); sl = step slot (0/1) in the
                2-step lg pair tile. M=32 (padded wout) so the 4 parities
                cover all 128 partitions."""
                out = lg_ps[32 * par:32 * par + 32, sl, :]
                for k in range(KC):
                    nc.tensor.matmul(
                        out, lhsT=wout_sb[:, k, :], rhs=h_new[:, k, :],
                        start=(k == 0), stop=(k == KC - 1) and not use_bout,
                        tile_position=(0, 32 * par))
                if use_bout:
                    nc.tensor.matmul(
                        out, lhsT=bout_sb, rhs=ones_row, start=False, stop=True,
                        tile_position=(0, 32 * par))

            def logits_out(t, g, lg_ps):
                """Evacuate one 2-step x 4-parity lg pair tile (steps t-1, t)."""
                lg_sb = g_pool.tile([128, 2, n_b], DT, tag="lgsb", bufs=2,
                                    name="lg_sb")
                nc.scalar.copy(lg_sb, lg_ps)
                for gq in range(n_par):
                    cb = (g * n_par + gq) * n_b
                    nc.sync.dma_start(
                        outT[t - 1:t + 1, :, cb:cb + n_b].rearrange(
                            "s v b -> v s b")[0:NTOK],
                        lg_sb[32 * gq:32 * gq + NTOK, :, :])

            # ---- main loop: one continuous software pipeline over all
            # (round, step) wall-slots; parity par works chunk 4*round+par.
            # Logits matmuls are deferred 2 parity-slots so h_new is never on
            # the tensor queue's critical path; next round's chunk DMAs are
            # prefetched at step 5 of the current round. ---------------------
            states = [chunk_dmas(0, par) for par in range(n_par)]
            nxt = [None] * n_par
            gins = [[None] * (n_groups * L + 4) for _ in range(n_par)]
            for par in range(n_par):
                for s in (1, 2):
                    gins[par][s] = gin_dma(0, par, s)
            pend = []  # deferred logits mms: (ws, par, h_new)
            ew_pend = []  # deferred elementwise chains: (ws, par, sig, hn, h)
            lgs = {}

            def pop_logits():
                ws, ppar, ph = pend.pop(0)
                logits_mm(ws % 2, ppar, ph, lgs[ws // 2])
                if ppar == n_par - 1 and ws % 2 == 1:
                    logits_out(ws % L, ws // L, lgs[ws // 2])
                    del lgs[ws // 2]

            def pop_chain():
                ws, ppar, sig, hn, h = ew_pend.pop(0)
                h_new = chain_stage(ws % L, ppar, sig, hn, gins[ppar][ws], h)
                pend.append((ws, ppar, h_new))
                cs, oh, _ = states[ppar]
                states[ppar] = (cs, oh, h_new)

            for ws in range(n_groups * L):
                g, t = ws // L, ws % L
                if ws % 2 == 0:
                    lgs[ws // 2] = ps_lg.tile([128, 2, n_b], F32, tag="lg",
                                              name=f"lg{t}")
                for par in range(n_par):
                    cs, oh, h = states[par]
                    rp, zp, hn = step_mms(t, par, oh, h)
                    sig = sig_stage(t, par, rp, zp)
                    ew_pend.append((ws, par, sig, hn, h))
                    if len(ew_pend) > 1:
                        pop_chain()
                    if len(pend) > 2:
                        pop_logits()
                    s = ws + 3
                    if s < n_groups * L and s % L != 0:
                        gins[par][s] = gin_dma(s // L, par, s % L)
                    if t == 5 and g + 1 < n_groups:
                        nxt[par] = chunk_dmas(g + 1, par)
                if t == L - 1:
                    while ew_pend:
                        pop_chain()
                    states, nxt = nxt, [None] * n_par
            while ew_pend:
                pop_chain()
            while pend:
                pop_logits()

    nc.compile()
    return nc


def make_in_maps(latent_context, target_sequence, emb_table, W_ih, W_hh,
                 b_ih, b_hh, W_out, b_out, b_core=B_CORE):
    """Shard + lay out the inputs for each core. Host-side layout/lookup prep."""
    lat = np.asarray(latent_context, dtype=np.float32)
    tok = np.asarray(target_sequence)
    emb = np.asarray(emb_table, dtype=np.float64)
    W_ih = np.asarray(W_ih, dtype=np.float64)
    b_ih = np.asarray(b_ih, dtype=np.float64)
    b_hh = np.asarray(b_hh, dtype=np.float64)

    # gi table: gi[v] = emb[v] @ W_ih.T + b_ih (+ b_hh on the rz part)
    gi = emb @ W_ih.T + b_ih
    gi[:, :512] += b_hh[:512]
    gi = gi.astype(np.float32)

    girz = np.zeros((8, 512), np.float16)
    girz[:NTOK] = gi[:NTOK, :512].astype(np.float16)
    gist = np.ascontiguousarray(
        gi[START].reshape(6, 128).T.astype(np.float32))  # [128, 6]

    whhT = np.ascontiguousarray(
        np.asarray(W_hh, np.float32).T.astype(np.float16))
    woutT = np.zeros((LATENT, 32), np.float16)
    woutT[:, :NTOK] = np.asarray(W_out, np.float32).T.astype(np.float16)
    b_out = np.asarray(b_out, dtype=np.float32)

    latT_all = lat.astype(np.float16).T  # [256, B]
    # input tokens for steps 1..7 are tgt[:, 0..6]
    ti = tok[:, :L - 1].astype(np.int64)  # [B, 7]
    oh_all = (ti[:, :, None] == np.arange(8)).astype(np.float16)  # [B,7,8]
    oh_all = np.ascontiguousarray(oh_all.transpose(2, 1, 0))  # [8,7,B]
    gi_n16 = gi[:, 512:].astype(np.float16)  # [6, 256]
    gin_all = gi_n16[ti]  # [B, 7, 256]
    gin_all = np.ascontiguousarray(gin_all.transpose(1, 2, 0))  # [7,256,B]

    n_cores_eff = lat.shape[0] // b_core
    in_maps = []
    for i in range(n_cores_eff):
        sl = slice(i * b_core, (i + 1) * b_core)
        m = {
            "latT": np.ascontiguousarray(latT_all[:, sl]),
            "ohd": np.ascontiguousarray(oh_all[:, :, sl]),
            "gind": np.ascontiguousarray(gin_all[:, :, sl]),
            "girz": girz,
            "gist": gist,
            "whhT": whhT,
            "woutT": woutT,
        }
        if np.any(b_hh[512:]):
            m["bhhn"] = np.ascontiguousarray(
                b_hh[512:].reshape(1, LATENT).astype(np.float16))
        if np.any(b_out):
            bo = np.zeros((1, 32), np.float16)
            bo[0, :NTOK] = b_out.astype(np.float16)
            m["bout"] = bo
        in_maps.append(m)
    return in_maps


_PROGRAM_CACHE = {}


def _get_program(b_core, use_bhh_n, use_bout):
    key = (b_core, use_bhh_n, use_bout)
    if key not in _PROGRAM_CACHE:
        _PROGRAM_CACHE[key] = build_program(
            b_core=b_core, use_bhh_n=use_bhh_n, use_bout=use_bout)
    return _PROGRAM_CACHE[key]


def run(inputs, trace=False, b_core=B_CORE, **_ignored):
    in_maps = make_in_maps(b_core=b_core, **inputs)
    use_bhh_n = "bhhn" in in_maps[0]
    use_bout = "bout" in in_maps[0]
    nc = _get_program(b_core, use_bhh_n, use_bout)
    core_ids = list(range(len(in_maps)))
    res = bass_utils.run_bass_kernel_spmd(nc, in_maps, core_ids, trace=trace)
    outs = []
    for i in core_ids:
        o = res.results[i]["outT"]  # (L, NTOK, b_core) f16
        outs.append(np.ascontiguousarray(
            np.transpose(o, (2, 0, 1)).astype(np.float32)))
    return np.concatenate(outs, axis=0), res


def kernel(**inputs) -> np.ndarray:
    out, _ = run(inputs, trace=False)
    return out
